# revision 43
# baseline (speedup 1.0000x reference)
"""Causal self-attention Trainium2 kernel.

Full-input contract: kernel(x[4,2048,1024], w_qkv[1024,3072], w_proj[1024,1024])
-> [4,2048,1024] fp32.

Sharding (8 cores): batch (4) x head-group (2 groups of 8 heads).
Each core computes, for its (batch b, head-group g):
  - QKV^T projection for its 8 heads (tensor parallel on qkv columns)
  - causal attention for 8 heads, flash-style in S^T = K @ Q^T layout
  - partial out-projection (tensor parallel on proj rows)
Host sums the two partial Y contributions per batch (the "all-reduce").

Per-core layouts (T=2048, C=1024, D=64, 8 local heads):
  xtc[tch] [128, 8*512]  x[b].T packed (partition p, free (ci, t)) per t-chunk
  wq_{q,k,v} [128, 8*512] weight shards packed (partition p, free (ci, col))
  wp       [128, 4*1024] proj row shard packed (partition p, free (ci2, col))
  QT/KT  [512, 2048]   per-head-pair SBUF tiles [128, 2048]
  V      16 t-tiles [128, 520] = 8 x ([128,64] V_h | ones column)
  S^T    [k=128, q=512] psum tiles = K_tile @ Q_chunk  (2-head row-packed PE)
  E^T    exp(S^T/8) bf16, causal-masked via gpsimd affine_select
  O^T    [65, 512] psum = [V_h|1].T @ E^T  (row 64 = softmax denominators)
  Y      [2048, 1024] fp32 partial

Inputs arrive via ~14 DMAs in consumption order (v-weights and x chunk 0
quarter-interleaved first, then q/k weights, x chunk 1, proj weights,
x chunks 2-3) so the PE starts as soon as the first megabyte lands.
Softmax normalization: per-head denominators are copied from the PSUM
ones-row into rows 0/32/64/96 of a shared tile, reciprocal'd and cast on
the DVE, then broadcast across partitions with one constant-selector PE
matmul per head pair ([97,128] 0/1 lhsT) -- no GpSimd, no rank-1 spam.
Fillers run in two queues: V/QK prep (must flush at each chunk boundary)
feeds PE gaps first; out-proj work is banked forward so chunk 3, which
has no V/QK prep, still has ~120 filler steps to hide exp latency.
Output y is f16 (halves the drain bytes; quantization adds ~1e-4 rel).
"""

import numpy as np
import ml_dtypes

import concourse.bass as bass
import concourse.bacc as bacc
import concourse.mybir as mybir
import concourse.tile as tile
from concourse.bass_utils import run_bass_kernel_spmd

B, T, C = 4, 2048, 1024
NH, D = 16, 64
HL = NH // 2          # heads per core
QC = 512              # q chunk (psum free dim)
KT = 128              # k tile (psum partitions)
NQC = T // QC         # 4 q chunks
NCT = C // KT         # 8 contraction tiles of 128
BF16 = mybir.dt.bfloat16
F32 = mybir.dt.float32

_CACHE = {}


def _build_nc():
    nc = bacc.Bacc("TRN2", target_bir_lowering=False, debug=False)
    # packed inputs: free dim is (ci, col) within each 512-col chunk
    xtp = nc.dram_tensor("xtp", [128, NQC * NCT * QC], BF16, kind="ExternalInput")
    wqp = nc.dram_tensor("wqp", [128, 3 * NCT * QC], BF16, kind="ExternalInput")
    wpp = nc.dram_tensor("wpp", [128, 4 * C], BF16, kind="ExternalInput")
    y = nc.dram_tensor("y", [T, C], mybir.dt.float16, kind="ExternalOutput")

    with tile.TileContext(nc) as tc:
        with (
            tc.tile_pool(name="xt", bufs=1) as xt_pool,
            tc.tile_pool(name="wq", bufs=1) as wq_pool,
            tc.tile_pool(name="wp", bufs=1) as wp_pool,
            tc.tile_pool(name="qt", bufs=1) as qt_pool,
            tc.tile_pool(name="kt", bufs=1) as kt_pool,
            tc.tile_pool(name="vt", bufs=1) as vt_pool,
            tc.tile_pool(name="et", bufs=8) as et_pool,
            tc.tile_pool(name="mk", bufs=1) as mk_pool,
            tc.tile_pool(name="on", bufs=1) as on_pool,
            tc.tile_pool(name="ou", bufs=8) as ou_pool,
            tc.tile_pool(name="sm", bufs=4) as sm_pool,
            tc.tile_pool(name="ys", bufs=3) as ys_pool,
            tc.tile_pool(name="sp", bufs=2, space="PSUM") as s_psum,
            tc.tile_pool(name="op", bufs=2, space="PSUM") as o_psum,
            # shared double-buffered pool for QKV-gen, proj and tail-norm
            # broadcast: alternating banks so a lagging DVE drain copy never
            # stalls the next matmul group (WAR on a single bank).
            tc.tile_pool(name="fp", bufs=2, space="PSUM") as fill_psum,
        ):
            # ---- load inputs: 8 large DMAs in consumption order ----
            # Each SBUF tile matches its DRAM layout exactly (pure 2D copy).
            xt_sb = [xt_pool.tile([128, NCT * QC], BF16, name=f"xtc{t}")
                     for t in range(NQC)]
            wq_v = wq_pool.tile([128, NCT * QC], BF16, name="wqv")
            wq_q = wq_pool.tile([128, NCT * QC], BF16, name="wqq")
            wq_k = wq_pool.tile([128, NCT * QC], BF16, name="wqk")
            wp_sb = wp_pool.tile([128, 4 * C], BF16, name="wp")
            CW = NCT * QC
            QW_ = CW // 4
            for q4 in range(4):
                s = slice(q4 * QW_, (q4 + 1) * QW_)
                nc.sync.dma_start(wq_v[:, s], wqp[:, 2 * CW + q4 * QW_:
                                                 2 * CW + (q4 + 1) * QW_])
                nc.sync.dma_start(xt_sb[0][:, s], xtp[:, s])
            nc.sync.dma_start(wq_q[:], wqp[:, 0:CW])
            nc.sync.dma_start(wq_k[:], wqp[:, CW:2 * CW])
            nc.sync.dma_start(xt_sb[1][:], xtp[:, CW:2 * CW])
            nc.sync.dma_start(wp_sb[:], wpp[:])
            nc.sync.dma_start(xt_sb[2][:], xtp[:, 2 * CW:3 * CW])
            nc.sync.dma_start(xt_sb[3][:], xtp[:, 3 * CW:4 * CW])

            def xt_ci(tch, ci):
                # [128, 512] view of x^T rows 128ci..128ci+127, cols of chunk
                return xt_sb[tch][:].rearrange(
                    "p (ci c) -> p ci c", c=QC)[:, ci, :]

            # ---- phase 1b: V tiles [128, 520] with ones columns,
            # emitted per-chunk below (chunk j needs tiles 4j..4j+3)
            v_sb = [None] * (T // KT)

            # QKV-projection work is generated as "filler steps" (one PE
            # matmul or one epilogue per step) so it can be drip-fed between
            # attention k-tile iterations, filling PE gaps while ACT (exp)
            # is the local bottleneck. fillers: deque of thunks.
            from collections import deque
            # Two filler queues: vq holds V/QK prep (must complete by the
            # next chunk boundary -- flushed there), pq holds out-proj work
            # (no deadline -- banked forward so chunk 3, which has no V/QK
            # prep of its own, still has filler mass to hide exp latency).
            vq = deque()
            pq = deque()
            fillers = vq  # gen_v/gen_qk append here
            # during a boundary flush the DVE is clogged with the norm
            # chain while Scalar sits idle; route drain copies accordingly.
            state = {"flush": False}

            def drain_copy(dst, src):
                if state["flush"]:
                    nc.scalar.copy(dst, src)
                else:
                    nc.vector.tensor_copy(dst, src)

            def feed(n, keep=0):
                for _ in range(n):
                    if vq:
                        vq.popleft()()
                    elif len(pq) > keep:
                        pq.popleft()()
                    else:
                        break

            def flush():
                # boundary flush: vq must drain; pq is deliberately kept.
                state["flush"] = True
                while vq:
                    vq.popleft()()
                state["flush"] = False

            def flush_all():
                state["flush"] = True
                while vq:
                    vq.popleft()()
                while pq:
                    pq.popleft()()
                state["flush"] = False

            def flush_n(n):
                for _ in range(min(n, len(vq))):
                    vq.popleft()()

            def gen_v(tt):
                tch, sub = tt // 4, tt % 4
                vt = vt_pool.tile([128, HL * (D + 1)], BF16, name=f"vt{tt}")
                ps = fill_psum.tile([128, QC], F32, name="ps", tag="qp")
                for ci in range(NCT):
                    fillers.append(
                        lambda ci=ci, ps=ps, tch=tch, sub=sub: nc.tensor.matmul(
                            ps[:],
                            lhsT=xt_ci(tch, ci)[:, sub * 128:(sub + 1) * 128],
                            rhs=wq_v[:].rearrange(
                                "p (ci c) -> p ci c", c=QC)[:, ci, :],
                            start=(ci == 0),
                            stop=(ci == NCT - 1),
                        )
                    )

                def tail(vt=vt, ps=ps, tt=tt):
                    drain_copy(
                        vt[:].rearrange("p (h e) -> p h e", e=D + 1)[:, :, 0:D],
                        ps[:].rearrange("p (h e) -> p h e", e=D),
                    )
                    nc.gpsimd.memset(
                        vt[:].rearrange("p (h e) -> p h e", e=D + 1)
                        [:, :, D:D + 1],
                        1.0,
                    )
                fillers.append(tail)
                v_sb[tt] = vt

            # ---- phase 1a: Q^T, K^T  [512,2048] each as 4 pair-tiles ----
            qt_sb = [qt_pool.tile([128, T], BF16, name=f"qt{i}") for i in range(4)]
            kt_sb = [kt_pool.tile([128, T], BF16, name=f"kts{i}") for i in range(4)]

            def gen_qk(tch):
                # pair-interleaved order (pair p needs qt[p] AND kt[p]) so
                # pair 0's attention can start as early as possible after the
                # flush; copies go on the Scalar engine, which is idle during
                # the flush window (the DVE is the congested one).
                for ct in (0, 4, 1, 5, 2, 6, 3, 7):
                    dst = qt_sb[ct] if ct < 4 else kt_sb[ct - 4]
                    wsrc = wq_q if ct < 4 else wq_k
                    cs = (ct % 4) * 128
                    ps = fill_psum.tile([128, QC], F32, name="ps", tag="qp")
                    for ci in range(NCT):
                        fillers.append(
                            lambda ci=ci, ps=ps, wsrc=wsrc, cs=cs, tch=tch:
                            nc.tensor.matmul(
                                ps[:],
                                lhsT=wsrc[:].rearrange(
                                    "p (ci c) -> p ci c", c=QC)
                                [:, ci, cs:cs + 128],
                                rhs=xt_ci(tch, ci),
                                start=(ci == 0),
                                stop=(ci == NCT - 1),
                            )
                        )
                    # mid-attention drains go on the DVE (Scalar runs the
                    # exp chain); boundary-flush drains go on Scalar.
                    fillers.append(
                        lambda dst=dst, ps=ps: drain_copy(
                            dst[:, tch * QC:(tch + 1) * QC], ps[:]
                        )
                    )

            # ---- phase 2: attention + out-proj, per q-chunk ----
            # Heads run per pair (2 heads sharing a QT/KT tile). Per k-tile,
            # both heads' S^T land in one [128,1024] PSUM tile (row-packed
            # concurrent matmuls via tile_position), one 1024-wide exp, then
            # two AV accumulations. O^T leaves PSUM immediately (unnormal-
            # ized); normalization happens later from SBUF.
            # With the q>=128m column restriction, only the 128-wide diagonal
            # BLOCK of each diag tile is partially masked (cols beyond it are
            # fully unmasked) -- and that block's triangle (keep q'' >= k) is
            # identical for every m. One [128, 2*128] mask [tri|tri] serves
            # all diagonal tiles and both head slots.
            mask_sb = mk_pool.tile([128, 2 * KT], BF16, name="masktri")
            nc.gpsimd.memset(mask_sb[:], 1.0)
            for half in range(2):
                nc.gpsimd.affine_select(
                    out=mask_sb[:, half * KT:(half + 1) * KT],
                    in_=mask_sb[:, half * KT:(half + 1) * KT],
                    compare_op=mybir.AluOpType.is_ge,
                    fill=0.0,
                    base=0,
                    pattern=[[1, KT]],
                    channel_multiplier=-1,
                )

            # selector matrices for the softmax-recip broadcast: one
            # matmul per [128, QC] bc tile (out row m of sub s takes head
            # 2s + m//64's reciprocal): bc = sel^T @ recip_rows. Partition
            # bases must be multiples of 32, so the recip rows live at
            # partitions 0/32/64/96 and sel has indicator rows there.
            sel_sb = [mk_pool.tile([97, 128], BF16, name=f"sel{s}")
                      for s in range(2)]
            for s in range(2):
                nc.gpsimd.memset(sel_sb[s][:], 0.0)
                for s2 in range(2):
                    hh = 2 * s + s2
                    nc.gpsimd.memset(
                        sel_sb[s][32 * hh:32 * hh + 1,
                                  s2 * 64:(s2 + 1) * 64],
                        1.0,
                    )

            on_sb = [on_pool.tile([128, T], BF16, name=f"on{i}") for i in range(4)]

            def emit_skt(j, pair, kt_i, nk, ops):
                # Diagonal tile m: columns q < 128m are fully causal-masked.
                # Skip them in the S matmul, exp AND the AV accumulation
                # (those O^T psum columns already have their full sum from
                # earlier k-tiles; stale et data there is never read).
                m = kt_i - 4 * j
                qoff = KT * m if m > 0 else 0
                sp = s_psum.tile([128, 2 * QC], F32, name="sp", tag="sp")
                for slot in range(2):
                    po = slot * 64
                    nc.tensor.matmul(
                        sp[:, slot * QC + qoff:(slot + 1) * QC],
                        lhsT=kt_sb[pair][po:po + 64, kt_i * KT:(kt_i + 1) * KT],
                        rhs=qt_sb[pair][po:po + 64, j * QC + qoff:(j + 1) * QC],
                        start=True,
                        stop=True,
                        tile_position=(po, 0),
                    )
                et = et_pool.tile([128, 2 * QC], BF16, name="et")
                if qoff:
                    nc.scalar.activation(
                        et[:].rearrange("p (s q) -> p s q", q=QC)[:, :, qoff:QC],
                        sp[:].rearrange("p (s q) -> p s q", q=QC)[:, :, qoff:QC],
                        mybir.ActivationFunctionType.Exp, scale=0.125,
                    )
                else:
                    nc.scalar.activation(
                        et[:], sp[:], mybir.ActivationFunctionType.Exp,
                        scale=0.125,
                    )
                if m >= 0:  # diagonal-crossing tile: mask the 128-col block
                    blk = (
                        et[:].rearrange("p (s q) -> p s q", q=QC)
                        [:, :, qoff:qoff + KT]
                    )
                    nc.vector.tensor_mul(
                        blk,
                        blk,
                        mask_sb[:].rearrange("p (s q) -> p s q", q=KT),
                    )
                # fill the PE wait for exp(kt) with queued QKV/proj matmuls;
                # in the last chunk hold back ~24 steps so the PE stays busy
                # through the final norm chain (avoids a HAM re-throttle).
                if j == 0:
                    # chunk 0: drain the remaining QK groups (18 thunks per
                    # pair) between iterations so pair p+1's Q/K is in SBUF
                    # by the time its S matmuls issue.
                    feed(5)
                elif j == NQC - 1:
                    # ~120 banked proj thunks over 64 iterations; diagonal
                    # k-tiles have shorter S/AV matmuls against the same
                    # exp latency, so weight the feed toward them.
                    feed(2 if kt_i < 4 * j else 3)
                else:
                    feed(2 if kt_i < 2 or kt_i >= 4 * j else 1)
                for slot in range(2):
                    h = pair * 2 + slot
                    nc.tensor.matmul(
                        ops[slot][:, qoff:QC] if qoff else ops[slot][:],
                        lhsT=v_sb[kt_i][:, h * (D + 1):(h + 1) * (D + 1)],
                        rhs=et[:, slot * QC + qoff:(slot + 1) * QC],
                        start=(kt_i == 0),
                        stop=(kt_i == nk - 1),
                    )

            def emit_pair(j, pair, nk, sums4, ou_t, tail=False):
                ops = [o_psum.tile([65, QC], F32, name=f"op{s}", tag="op")
                       for s in range(2)]
                for kt_i in range(nk):
                    emit_skt(j, pair, kt_i, nk, ops)
                # both heads of the pair land in one [128, QC] tile so one
                # norm multiply covers the pair (matches on_sb layout).
                # For the very last pair the drain copies go on the Scalar
                # engine (idle after the last exp) to shorten the tail chain.
                ou2 = ou_pool.tile([128, QC], BF16, name="ou2")
                for slot in range(2):
                    hh = (pair % 2) * 2 + slot
                    # denominator row straight from PSUM into this norm
                    # group's pooled sums tile (rows 0/32/64/96).
                    nc.vector.tensor_copy(
                        sums4[32 * hh:32 * hh + 1, :],
                        ops[slot][64:65, :],
                    )
                    if tail:
                        nc.scalar.copy(
                            ou2[slot * 64:(slot + 1) * 64, :],
                            ops[slot][0:64, :],
                        )
                    else:
                        nc.vector.tensor_copy(
                            ou2[slot * 64:(slot + 1) * 64, :],
                            ops[slot][0:64, :],
                        )
                ou_t.append(ou2)

            def emit_norm(j, half4, sums4, ou_t, pre_feed=0):
                # 1/sums on the DVE (recip reads the SBUF sums tile; the
                # non-data rows are finite 1.0s from the per-group memset),
                # then one full-tile bf16 cast for the PE broadcast matmul.
                recip4 = sm_pool.tile([97, QC], F32, name="recip4", tag="recip")
                nc.vector.reciprocal_approx_fast(out=recip4[:], in_=sums4[:])
                rc4b = sm_pool.tile([97, QC], BF16, name="rc4b", tag="rc4b")
                nc.vector.tensor_copy(rc4b[:], recip4[:])
                # keep the PE fed while the DVE produces the recip rows
                if pre_feed < 0:
                    flush_all()
                else:
                    feed(pre_feed)
                # Broadcast 1/sum across 64 partitions per head with ONE
                # selector matmul per [128, QC] bc tile: keeps the chain
                # off GpSimd and out of the DVE FIFO.
                for sub in range(2):  # one o_psum bank per head pair
                    bc = o_psum.tile([128, QC], F32, name="bcp", tag="op")
                    nc.tensor.matmul(
                        bc[:], lhsT=sel_sb[sub][:], rhs=rc4b[:],
                        start=True, stop=True,
                    )
                    nc.vector.tensor_mul(
                        on_sb[half4 * 2 + sub][:, j * QC:(j + 1) * QC],
                        ou_t[sub][:],
                        bc[:],
                    )

            def gen_proj(j, qq_range):
                last = j == NQC - 1
                for qq in qq_range:
                    qt0 = j * QC + qq * 128
                    for co in range(2):
                        yp = fill_psum.tile([128, QC], F32, name="yp", tag="qp")
                        for ci2 in range(4):
                            pq.append(
                                lambda yp=yp, ci2=ci2, qt0=qt0, co=co:
                                nc.tensor.matmul(
                                    yp[:],
                                    lhsT=on_sb[ci2][:, qt0:qt0 + 128],
                                    rhs=wp_sb[:].rearrange(
                                        "p (ci c) -> p ci c", c=C)
                                    [:, ci2, co * QC:(co + 1) * QC],
                                    start=(ci2 == 0),
                                    stop=(ci2 == 3),
                                )
                            )

                        def tail(yp=yp, qt0=qt0, co=co, qq=qq, last=last):
                            yst = ys_pool.tile([128, QC], mybir.dt.float16, name="yst")
                            # last chunk: Scalar is idle after the final exp,
                            # so draining there overlaps the DVE norm chain;
                            # spread the DMA issues across engines so the
                            # transfers land on independent queues.
                            if last:
                                nc.scalar.copy(yst[:], yp[:])
                                eng = (nc.sync, nc.scalar,
                                       nc.gpsimd)[(2 * qq + co) % 3]
                            else:
                                drain_copy(yst[:], yp[:])
                                eng = nc.sync
                            eng.dma_start(
                                y[qt0:qt0 + 128, co * QC:(co + 1) * QC],
                                yst[:],
                            )
                        pq.append(tail)

            for j in range(NQC):
                if j == 0:
                    for tt in range(4):
                        gen_v(tt)
                    gen_qk(0)
                    # flush V (4x9 thunks) + the first pair's QK groups
                    # (ct 0 and 4: 2x9 thunks); the remaining 6 QK groups
                    # drain as fillers during chunk-0 attention.
                    flush_n(4 * 9 + 2 * 9)
                # queue next chunk's V/QK as PE-gap fillers for this chunk
                if j < NQC - 1:
                    for tt in range(4 * j + 4, 4 * j + 8):
                        gen_v(tt)
                    gen_qk(j + 1)
                nk = 4 * j + 4  # causal: k tiles 0..nk-1
                for half4 in range(2):  # two groups of 2 pairs each
                    sums4 = sm_pool.tile([97, QC], F32, name="sums4",
                                         tag="sums")
                    # non-data rows must stay finite through the recip (the
                    # zero selector rows would turn NaN into NaN in the bc
                    # matmul); GpSimd is idle, so pre-fill with 1.0.
                    nc.gpsimd.memset(sums4[:], 1.0)
                    ou_t = []
                    for pp in range(2):
                        emit_pair(j, half4 * 2 + pp, nk, sums4, ou_t,
                                  tail=(j == NQC - 1 and half4 == 1))
                    # reserved fillers go into the PE FIFO *after* the
                    # tail-norm recip chain is issued but *before* the bc
                    # matmuls, so the PE stays busy while the DVE finishes.
                    emit_norm(j, half4, sums4, ou_t,
                              pre_feed=(-1 if j == NQC - 1 and half4 == 1
                                        else 9))
                if j < NQC - 1:
                    flush()  # qkv(j+1) must complete before attention(j+1)
                gen_proj(j, range(0, 4))
            flush_all()
    nc.finalize()
    return nc


def _shard_inputs(x, w_qkv, w_proj):
    bf = ml_dtypes.bfloat16
    in_maps = []
    for core in range(8):
        b, g = core // 2, core % 2
        cols = slice(g * HL * D, (g + 1) * HL * D)
        # wqp: per section s in (q,k,v): [1024, 512] -> [128, (ci, c)]
        secs = []
        for s in range(3):
            ws = w_qkv[:, s * C:(s + 1) * C][:, cols]          # [1024, 512]
            secs.append(
                ws.reshape(NCT, 128, QC).transpose(1, 0, 2).reshape(128, -1)
            )
        wqp = np.ascontiguousarray(np.concatenate(secs, axis=1).astype(bf))
        # wpp: [512, 1024] -> [128, (ci2, c)]
        wp = w_proj[g * HL * D:(g + 1) * HL * D, :]
        wpp = np.ascontiguousarray(
            wp.reshape(4, 128, C).transpose(1, 0, 2).reshape(128, -1).astype(bf)
        )
        # xtp: x[b].T [1024, 2048] -> [128, (tch, ci, c)]
        xt = x[b].T.reshape(NCT, 128, NQC, QC).transpose(1, 2, 0, 3)
        xtp = np.ascontiguousarray(xt.reshape(128, -1).astype(bf))
        in_maps.append({"xtp": xtp, "wqp": wqp, "wpp": wpp})
    return in_maps


def kernel(x, w_qkv, w_proj, trace=False, **trace_kwargs):
    if "nc" not in _CACHE:
        _CACHE["nc"] = _build_nc()
    nc = _CACHE["nc"]
    in_maps = _shard_inputs(
        np.asarray(x, np.float32), np.asarray(w_qkv, np.float32),
        np.asarray(w_proj, np.float32)
    )
    res = run_bass_kernel_spmd(
        nc, in_maps, core_ids=list(range(8)), trace=trace, **trace_kwargs
    )
    parts = [res.results[core]["y"] for core in range(8)]
    out = np.stack(
        [parts[2 * b].astype(np.float32) + parts[2 * b + 1].astype(np.float32)
         for b in range(B)]
    )
    if trace:
        _CACHE["last_result"] = res
    return out
